# revision 1
# baseline (speedup 1.0000x reference)
"""Trainium2 Bass kernel for nn_DebedderNeuronGroup_index.

Math (per layer l, with kn=KN[l], ksci=KS[l]*CI[l], i_dim=ksci+1):
    out[b, k, o] = sum_d x[b, off_l + k, d] * W_l[o, d] + b_l[o]
    y[b, S_l + k*ksci + o] = out[b, k, o]          for o <  ksci
    y[b, S_l + kn*ksci + k] = out[b, k, ksci]      (bias column tail block)
The five layers' outputs exactly tile y's 1,422,218 columns, so every
element of y is written exactly once (pure permutation, no accumulation).

Strategy: pure data parallelism over batch (16 per core, 8 cores).
Host pre-transposes x to xT[d, token] (token order layer-major then
batch-major) and W to WT[d, o], both cast to bf16 (matmul runs 4x faster
than fp32 on the PE; rel err ~5e-4). On device, per 128-token tile:
tokens sit on PSUM partitions (stationary operand = xT tile), o on the
free dim, so every HBM store is a [tokens, o] tile whose rows are
contiguous runs in y. Bias is added during the PSUM->SBUF drain with a
host-replicated [128, ksci] broadcast table. The bias column (o == ksci)
is computed in a separate tiny pass with M=1 matmuls producing [1, token]
rows that store contiguously into the tail blocks.
"""

import numpy as np
import ml_dtypes

import concourse.bass as bass
import concourse.mybir as mybir
from concourse import bacc
from concourse.tile import TileContext
from concourse.bass_utils import run_bass_kernel_spmd

# ---------------------------------------------------------------- constants
N_CORES = 8
B = 128
BPC = B // N_CORES            # batches per core = 16
D = 512
KN = [64, 128, 256, 256, 10]
KSCI = [27, 576, 1152, 4096, 256]
IDIM = [k + 1 for k in KSCI]
START = [0, 1792, 75648, 370816, 1419648]
I_TOTAL = 1422218
TOK = sum(KN)                 # 714 tokens per batch
TOKL = [BPC * k for k in KN]  # tokens per core per layer
XOFF = np.cumsum([0] + TOKL).tolist()   # token offset per layer in xT
NTOK = XOFF[-1]               # 11424
BBOFF = np.cumsum([0] + KSCI).tolist()  # bias-broadcast offset per layer
BBTOT = BBOFF[-1]             # 6107
TLOAD = 1024                  # tokens per x DMA chunk
OTILE = 512                   # matmul moving free dim / PSUM bank
BF16 = mybir.dt.bfloat16
F16 = mybir.dt.float16
F32 = mybir.dt.float32

_cache = {}
last_results = None


def _build_bass():
    nc = bacc.Bacc(
        "TRN2", target_bir_lowering=False, debug=False, num_devices=N_CORES
    )
    xT = nc.declare_dram_parameter("xT", [D, NTOK], BF16, isOutput=False)
    WT = [
        nc.declare_dram_parameter(f"WT{l}", [D, IDIM[l]], BF16, isOutput=False)
        for l in range(5)
    ]
    BB = nc.declare_dram_parameter("BB", [128, BBTOT], BF16, isOutput=False)
    BCOL = nc.declare_dram_parameter("BCOL", [1, 8], F32, isOutput=False)
    y = nc.declare_dram_parameter("y", [BPC, I_TOTAL], F16, isOutput=True)

    xT3 = xT[:, :].rearrange("(c p) t -> p c t", p=128)      # [128, 4, NTOK]

    with TileContext(nc) as tc:
        with (
            tc.tile_pool(name="wt", bufs=1) as wt_pool,
            tc.tile_pool(name="bias", bufs=1) as bias_pool,
            tc.tile_pool(name="x", bufs=4) as x_pool,
            tc.tile_pool(name="out", bufs=4) as out_pool,
            tc.tile_pool(name="ocol", bufs=4) as ocol_pool,
            tc.tile_pool(name="ps", bufs=6, space="PSUM") as ps_pool,
            tc.tile_pool(name="pscol", bufs=2, space="PSUM") as pscol_pool,
        ):
            # Tables are loaded just-in-time per layer (first matmul would
            # otherwise stall ~35us behind 9.4 MB of upfront table DMAs).
            bb = bias_pool.tile([128, BBTOT], BF16, tag="bb")
            bcol = bias_pool.tile([1, 8], F32, tag="bcol")
            nc.gpsimd.dma_start(out=bcol[:], in_=BCOL[:, :])

            # Layer 4 early (its tiny strided stores hide under compute);
            # layer 3 last (largest, most efficient stores stream the tail).
            SEQ = [0, 4, 1, 2, 3]

            def load_tables(l):
                t = wt_pool.tile([128, 4 * IDIM[l]], BF16, tag=f"wt{l}")
                t3 = t[:].rearrange("p (c o) -> p c o", c=4)
                nc.gpsimd.dma_start(
                    out=t3, in_=WT[l][:, :].rearrange("(c p) o -> p c o", p=128)
                )
                nc.gpsimd.dma_start(
                    out=bb[:, BBOFF[l] : BBOFF[l] + KSCI[l]],
                    in_=BB[:, BBOFF[l] : BBOFF[l] + KSCI[l]],
                )
                return t3

            # All tables load upfront on the SWDGE ring (separate from the
            # x-load SP ring), queued in processing order: each layer's
            # tables land before that layer's first matmul while the SP
            # ring streams x uncontended.
            wt3_by_layer = {l: load_tables(l) for l in SEQ}
            for li, l in enumerate(SEQ):
                wt3_l = wt3_by_layer[l]
                kn, ksci = KN[l], KSCI[l]
                # y main region viewed [b, k, o]; tail region viewed [b, k]
                y_main = y[:, START[l] : START[l] + kn * ksci].rearrange(
                    "b (k o) -> b k o", o=ksci
                )
                y_col = y[:, START[l] + kn * ksci : START[l] + kn * ksci + kn]
                # subtile = whole batches when kn < 128, else 128-token slice
                ts = 128 if kn >= 128 else (128 // kn) * kn
                for t0 in range(0, TOKL[l], TLOAD):
                    tl = min(TLOAD, TOKL[l] - t0)
                    xt = x_pool.tile([128, 4 * TLOAD], BF16, tag="xt")
                    xt3 = xt[:].rearrange("p (c t) -> p c t", c=4)
                    nc.sync.dma_start(
                        out=xt3[:, :, :tl],
                        in_=xT3[:, :, XOFF[l] + t0 : XOFF[l] + t0 + tl],
                    )
                    # ---- main pass: tokens on partitions, o on free dim.
                    # All o-tiles of a token-subtile drain into one wide SBUF
                    # tile so each store DMA writes full ksci-long rows
                    # (8 KB runs for layer 3 instead of 1 KB per o-tile).
                    for s0 in range(0, tl, ts):
                        sl = min(ts, tl - s0)         # tokens in subtile
                        tok = t0 + s0                  # layer-token index
                        b0 = tok // kn                 # first batch
                        nb = max(1, sl // kn)          # batches in subtile
                        k0 = tok - b0 * kn             # first k (0 unless kn>128... )
                        ob = out_pool.tile([128, 4096], F16, tag="ob")
                        for o0 in range(0, ksci, OTILE):
                            no = min(OTILE, ksci - o0)
                            ps = ps_pool.tile([128, OTILE], F32, tag="ps")
                            for dc in range(4):
                                nc.tensor.matmul(
                                    out=ps[:sl, :no],
                                    lhsT=xt3[:, dc, s0 : s0 + sl],
                                    rhs=wt3_l[:, dc, o0 : o0 + no],
                                    start=(dc == 0),
                                    stop=(dc == 3),
                                )
                            nc.any.tensor_add(
                                out=ob[:sl, o0 : o0 + no],
                                in0=ps[:sl, :no],
                                in1=bb[:sl, BBOFF[l] + o0 : BBOFF[l] + o0 + no],
                            )
                        # store per batch: [nk, ksci] rows contiguous in y
                        nk = min(kn, sl)
                        for bi in range(nb):
                            nc.scalar.dma_start(
                                out=y_main[b0 + bi, k0 : k0 + nk, :],
                                in_=ob[bi * nk : bi * nk + nk, :ksci],
                            )
                    # ---- bias-column pass: [1, token] rows
                    for c0 in range(0, tl, OTILE):
                        cl = min(OTILE, tl - c0)
                        pc = pscol_pool.tile([1, OTILE], F32, tag="pc")
                        for dc in range(4):
                            nc.tensor.matmul(
                                out=pc[:1, :cl],
                                lhsT=wt3_l[:, dc, ksci : ksci + 1],
                                rhs=xt3[:, dc, c0 : c0 + cl],
                                start=(dc == 0),
                                stop=(dc == 3),
                            )
                        oc = ocol_pool.tile([1, OTILE], F16, tag="oc")
                        nc.any.tensor_scalar_add(
                            out=oc[:1, :cl],
                            in0=pc[:1, :cl],
                            scalar1=bcol[0:1, l : l + 1],
                        )
                        # tokens (t0+c0 .. +cl) are whole batches here
                        cb0 = (t0 + c0) // kn
                        cnb = cl // kn
                        for bi in range(cnb):
                            nc.gpsimd.dma_start(
                                out=y_col[cb0 + bi, :],
                                in_=oc[0:1, bi * kn : (bi + 1) * kn],
                            )
    nc.compile()
    return nc


def _prep_inputs(inputs):
    x = np.asarray(inputs["x"], dtype=np.float32)
    xb = x.astype(ml_dtypes.bfloat16)
    in_maps = []
    # shared across cores
    shared = {}
    for l in range(5):
        W = np.asarray(inputs[f"W{l}"], dtype=np.float32)
        shared[f"WT{l}"] = np.ascontiguousarray(W.astype(ml_dtypes.bfloat16).T)
    bbvec = np.concatenate(
        [np.asarray(inputs[f"b{l}"], dtype=np.float32)[: KSCI[l]] for l in range(5)]
    )
    shared["BB"] = np.ascontiguousarray(
        np.broadcast_to(bbvec.astype(ml_dtypes.bfloat16), (128, BBTOT))
    )
    bcol = np.zeros((1, 8), np.float32)
    for l in range(5):
        bcol[0, l] = np.asarray(inputs[f"b{l}"], dtype=np.float32)[KSCI[l]]
    shared["BCOL"] = bcol
    off = np.cumsum([0] + KN).tolist()
    for c in range(N_CORES):
        xc = xb[c * BPC : (c + 1) * BPC]  # [16, 714, 512] bf16
        parts = [
            np.transpose(xc[:, off[l] : off[l] + KN[l]], (2, 0, 1)).reshape(D, -1)
            for l in range(5)
        ]
        xT = np.ascontiguousarray(np.concatenate(parts, axis=1))  # [512, 11424]
        in_maps.append({"xT": xT, **shared})
    return in_maps


def kernel(**inputs):
    global last_results
    if "nc" not in _cache:
        _cache["nc"] = _build_bass()
    nc = _cache["nc"]
    in_maps = _prep_inputs(inputs)
    res = run_bass_kernel_spmd(nc, in_maps, list(range(N_CORES)))
    last_results = res
    y = np.concatenate(
        [res.results[c]["y"].astype(np.float32) for c in range(N_CORES)], axis=0
    )
    return y



# revision 6
# speedup vs baseline: 1.1229x; 1.1229x over previous
"""Trainium2 Bass kernel for nn_DebedderNeuronGroup_index.

Math (per layer l, kn=KN[l], ksci=KS[l]*CI[l], idim=ksci+1):
    out[b, k, o] = sum_d x[b, off_l + k, d] * W_l[o, d] + b_l[o]
    y[b, S_l + k*ksci + o] = out[b, k, o]        for o <  ksci
    y[b, S_l + kn*ksci + k] = out[b, k, ksci]    (bias-column tail block)

Strategy: pure data parallelism over batch (16 per core, 8 cores), bf16
matmuls (tokens stationary on PSUM partitions, o on the free dim), f16
stores. v2 scheduling, built from the v1 trace:
  - x is staged in HBM chunk-major ([128, 4*tl] per 1024-token chunk, c-major
    free dim) so every chunk load is 128 descriptors of 8KB instead of 512 of
    2KB; v1's 2KB descriptors capped the x queue at ~78GB/s and starved the
    PE for ~40us during L1/L2.
  - The bias column is folded into the main o-tiling (o-tiles cover idim =
    ksci+1, equal-split <=512), removing v1's separate column pass (~19us of
    PE streaming).  Per-subtile bias-column values are extracted to a
    [128, 80] SBUF buffer (scalar-engine copies), stored once, and scattered
    into y on the host (layout-only work).
  - L0/L4 outputs go to small DRAM scratches whole (host scatters them);
    their y-layout stores were ~1200 tiny (54-512B) descriptors that clog a
    DMA queue for ~25us.
  - L2 and L3 subtiles are interleaved 1:1 so the store stream is smooth
    (139GB/s) instead of a 378GB/s burst (L2) followed by 152GB/s (L3);
    stores are split across the scalar-ring queue (L1/L2) and gpsimd-ring
    queue (L3 halves), with WT3 preloaded on the vector-ring queue.
  - ~120 warmup matmuls on a zeroed tile keep the PE busy during the initial
    x DMA so the HAM clock gate ramps to full speed before real work.
"""

import numpy as np
import ml_dtypes

import concourse.bass as bass
import concourse.mybir as mybir
from concourse import bacc
from concourse.tile import TileContext
from concourse.bass_utils import run_bass_kernel_spmd

# ---------------------------------------------------------------- constants
N_CORES = 8
B = 128
BPC = B // N_CORES            # batches per core = 16
D = 512
KN = [64, 128, 256, 256, 10]
KSCI = [27, 576, 1152, 4096, 256]
IDIM = [k + 1 for k in KSCI]
START = [0, 1792, 75648, 370816, 1419648]
I_TOTAL = 1422218
TOKL = [BPC * k for k in KN]  # tokens per core per layer
NTOK = sum(TOKL)              # 11424
BBOFF = np.cumsum([0] + IDIM).tolist()  # bias table offset per layer
BBTOT = BBOFF[-1]             # 6112
TLOAD = 1024                  # tokens per x DMA chunk
BF16 = mybir.dt.bfloat16
F16 = mybir.dt.float16
F32 = mybir.dt.float32

# o-tile widths per layer: equal split of idim into ceil(idim/512) tiles
def _otw(idim):
    nt = -(-idim // 512)
    base, rem = divmod(idim, nt)
    return [base + 1] * rem + [base] * (nt - rem)
OTW = [_otw(i) for i in IDIM]

# x chunk schedule: (layer, t0, tl) in issue order
CH = [(0, 0, 1024), (4, 0, 160), (1, 0, 1024), (1, 1024, 1024)]
for c in range(4):
    CH.append((2, c * 1024, 1024))
    CH.append((3, c * 1024, 1024))
XFOFF = np.cumsum([0] + [4 * tl for (_, _, tl) in CH]).tolist()
XFTOT = XFOFF[-1]             # 45696

# bias-column scratch layout: layer -> (col base, n subtiles)
COLBASE = {1: 0, 2: 16, 3: 48}
NCOLS = 80

_cache = {}
last_results = None


def _build_bass():
    nc = bacc.Bacc(
        "TRN2", target_bir_lowering=False, debug=False, num_devices=N_CORES
    )
    XF = nc.declare_dram_parameter("XF", [128, XFTOT], BF16, isOutput=False)
    WT = [
        nc.declare_dram_parameter(f"WT{l}", [128, 4 * IDIM[l]], BF16, isOutput=False)
        for l in range(5)
    ]
    BBp = nc.declare_dram_parameter("BB", [128, BBTOT], BF16, isOutput=False)
    y = nc.declare_dram_parameter("y", [BPC, I_TOTAL], F16, isOutput=True)
    Y0S = nc.declare_dram_parameter("Y0S", [128, 8 * 28], F16, isOutput=True)
    Y4S = nc.declare_dram_parameter("Y4S", [128, 2 * 257], F16, isOutput=True)
    YCOL = nc.declare_dram_parameter("YCOL", [128, NCOLS], F16, isOutput=True)

    with TileContext(nc) as tc:
        with (
            tc.tile_pool(name="wt", bufs=1) as wt_pool,
            tc.tile_pool(name="bias", bufs=1) as bias_pool,
            tc.tile_pool(name="x", bufs=5) as x_pool,
            tc.tile_pool(name="outL", bufs=4) as outL_pool,
            tc.tile_pool(name="out1", bufs=16) as out1_pool,
            tc.tile_pool(name="outS", bufs=8) as outS_pool,
            tc.tile_pool(name="fix", bufs=1) as fix_pool,
            tc.tile_pool(name="ps", bufs=6, space="PSUM") as ps_pool,
            tc.tile_pool(name="pw", bufs=1, space="PSUM") as pw_pool,
        ):
            # ---- PE warmup: keep the HAM activity monitor fed while the
            # first x chunk loads, so real matmuls start at full clock.
            warm = fix_pool.tile([128, 128], BF16, tag="warm")
            nc.vector.memset(warm[:, :], 0.0)
            wps = pw_pool.tile([128, 128], F32, tag="wps")
            for _ in range(120):
                nc.tensor.matmul(
                    out=wps[:, :], lhsT=warm[:, :], rhs=warm[:, :],
                    start=True, stop=True,
                )

            # ---- tables.  WT3 (4.2MB, needed ~35us in) gets the scalar-ring
            # queue (idle until stores begin); the rest stream on the gpsimd
            # ring in use order.
            bb = bias_pool.tile([128, BBTOT], BF16, tag="bb")
            wt = {}

            def load_tables(l, eng):
                t = wt_pool.tile([128, 4 * IDIM[l]], BF16, tag=f"wt{l}")
                eng.dma_start(out=t[:, :], in_=WT[l][:, :])
                eng.dma_start(
                    out=bb[:, BBOFF[l] : BBOFF[l] + IDIM[l]],
                    in_=BBp[:, BBOFF[l] : BBOFF[l] + IDIM[l]],
                )
                wt[l] = t[:, :].rearrange("p (c o) -> p c o", c=4)

            load_tables(3, nc.scalar)
            for l in (0, 4, 1, 2):
                load_tables(l, nc.gpsimd)

            colbuf = fix_pool.tile([128, NCOLS], F16, tag="colbuf")
            ob0 = fix_pool.tile([128, 8 * 28], F16, tag="ob0")
            ob4 = fix_pool.tile([128, 2 * 257], F16, tag="ob4")
            nc.vector.memset(ob4[:, :], 0.0)  # rows >= sl never written

            ymain = {
                l: y[:, START[l] : START[l] + KN[l] * KSCI[l]].rearrange(
                    "b (k o) -> b k o", o=KSCI[l]
                )
                for l in (1, 2, 3)
            }

            def load_chunk(ci):
                _, _, tl = CH[ci]
                xt = x_pool.tile([128, 4 * TLOAD], BF16, tag="xt")
                nc.sync.dma_start(
                    out=xt[:, : 4 * tl], in_=XF[:, XFOFF[ci] : XFOFF[ci] + 4 * tl]
                )
                return xt[:, : 4 * tl].rearrange("p (c t) -> p c t", c=4)

            def subtile_mm(l, xv, s0, sl, drain):
                o0 = 0
                for oi, no in enumerate(OTW[l]):
                    ps = ps_pool.tile([128, 512], F32, tag="ps")
                    for dc in range(4):
                        nc.tensor.matmul(
                            out=ps[:sl, :no],
                            lhsT=xv[:, dc, s0 : s0 + sl],
                            rhs=wt[l][:, dc, o0 : o0 + no],
                            start=(dc == 0),
                            stop=(dc == 3),
                        )
                    drain(oi, o0, no, ps)
                    o0 += no

            def badd(out_ap, ps, sl, no, l, o0):
                nc.vector.tensor_add(
                    out=out_ap,
                    in0=ps[:sl, :no],
                    in1=bb[:sl, BBOFF[l] + o0 : BBOFF[l] + o0 + no],
                )

            # ---- phase A: L0 -> scratch
            xv = load_chunk(0)
            for s in range(8):
                def dr0(oi, o0, no, ps, s=s):
                    badd(ob0[:128, s * 28 + o0 : s * 28 + o0 + no], ps, 128, no, 0, o0)
                subtile_mm(0, xv, s * 128, 128, dr0)
            nc.gpsimd.dma_start(out=Y0S[:, :], in_=ob0[:, :])

            # ---- L4 -> scratch (2 subtiles: 120 + 40 tokens)
            xv = load_chunk(1)
            for si, (s0, sl) in enumerate(((0, 120), (120, 40))):
                def dr4(oi, o0, no, ps, si=si, sl=sl):
                    badd(
                        ob4[:sl, si * 257 + o0 : si * 257 + o0 + no],
                        ps, sl, no, 4, o0,
                    )
                subtile_mm(4, xv, s0, sl, dr4)
            nc.gpsimd.dma_start(out=Y4S[:, :], in_=ob4[:, :])

            # ---- L1: 16 subtiles = 16 batches; stores alternate rings
            for ch in range(2):
                xv = load_chunk(2 + ch)
                for si in range(8):
                    s = ch * 8 + si
                    ob = out1_pool.tile([128, IDIM[1]], F16, tag="ob1")
                    def dr1(oi, o0, no, ps, ob=ob):
                        badd(ob[:128, o0 : o0 + no], ps, 128, no, 1, o0)
                    subtile_mm(1, xv, si * 128, 128, dr1)
                    eng = nc.scalar if s % 2 == 0 else nc.gpsimd
                    eng.dma_start(out=ymain[1][s, 0:128, :], in_=ob[:, : KSCI[1]])
                    nc.scalar.copy(
                        out=colbuf[:, COLBASE[1] + s : COLBASE[1] + s + 1],
                        in_=ob[:, KSCI[1] : IDIM[1]],
                    )

            # ---- phase B: L2 and L3 interleaved 1:1 (32 subtiles each)
            for ch in range(4):
                x2 = load_chunk(4 + 2 * ch)
                x3 = load_chunk(5 + 2 * ch)
                for si in range(8):
                    s = ch * 8 + si
                    b0, k0 = divmod(s * 128, KN[2])
                    # L2 subtile
                    ob = outS_pool.tile([128, IDIM[2]], F16, tag="ob2")
                    def dr2(oi, o0, no, ps, ob=ob):
                        badd(ob[:128, o0 : o0 + no], ps, 128, no, 2, o0)
                    subtile_mm(2, x2, si * 128, 128, dr2)
                    eng = nc.scalar if s % 2 == 0 else nc.gpsimd
                    eng.dma_start(
                        out=ymain[2][b0, k0 : k0 + 128, :], in_=ob[:, : KSCI[2]]
                    )
                    nc.scalar.copy(
                        out=colbuf[:, COLBASE[2] + s : COLBASE[2] + s + 1],
                        in_=ob[:, KSCI[2] : IDIM[2]],
                    )
                    # L3 subtile; store in halves so the queue flows smoothly
                    ob3 = outL_pool.tile([128, IDIM[3]], F16, tag="ob3")
                    def dr3(oi, o0, no, ps, ob3=ob3, b0=b0, k0=k0):
                        badd(ob3[:128, o0 : o0 + no], ps, 128, no, 3, o0)
                        if oi == 4:  # columns [0, 2277) drained, store [0, 2048)
                            nc.gpsimd.dma_start(
                                out=ymain[3][b0, k0 : k0 + 128, 0:2048],
                                in_=ob3[:, 0:2048],
                            )
                    subtile_mm(3, x3, si * 128, 128, dr3)
                    nc.gpsimd.dma_start(
                        out=ymain[3][b0, k0 : k0 + 128, 2048:4096],
                        in_=ob3[:, 2048:4096],
                    )
                    nc.scalar.copy(
                        out=colbuf[:, COLBASE[3] + s : COLBASE[3] + s + 1],
                        in_=ob3[:, KSCI[3] : IDIM[3]],
                    )
            nc.gpsimd.dma_start(out=YCOL[:, :], in_=colbuf[:, :])
    nc.compile()
    return nc


def _prep_inputs(inputs):
    x = np.asarray(inputs["x"], dtype=np.float32)
    xb = x.astype(ml_dtypes.bfloat16)
    shared = {}
    for l in range(5):
        W = np.asarray(inputs[f"W{l}"], dtype=np.float32).astype(ml_dtypes.bfloat16)
        # [128, 4*idim] with free dim (c, o); row p, chunk c holds W.T[c*128+p]
        shared[f"WT{l}"] = np.ascontiguousarray(
            W.T.reshape(4, 128, IDIM[l]).transpose(1, 0, 2).reshape(128, 4 * IDIM[l])
        )
    bbvec = np.concatenate(
        [np.asarray(inputs[f"b{l}"], dtype=np.float32)[: IDIM[l]] for l in range(5)]
    )
    shared["BB"] = np.ascontiguousarray(
        np.broadcast_to(bbvec.astype(ml_dtypes.bfloat16), (128, BBTOT))
    )
    off = np.cumsum([0] + KN).tolist()
    in_maps = []
    for c in range(N_CORES):
        xc = xb[c * BPC : (c + 1) * BPC]  # [16, 714, 512] bf16
        xTl = [
            np.transpose(xc[:, off[l] : off[l] + KN[l]], (2, 0, 1)).reshape(D, -1)
            for l in range(5)
        ]
        parts = []
        for l, t0, tl in CH:
            blk = xTl[l][:, t0 : t0 + tl]  # [512, tl]
            parts.append(
                blk.reshape(4, 128, tl).transpose(1, 0, 2).reshape(128, 4 * tl)
            )
        in_maps.append({"XF": np.ascontiguousarray(np.concatenate(parts, axis=1)),
                        **shared})
    return in_maps


def _assemble(res):
    y = np.empty((B, I_TOTAL), np.float32)
    for c in range(N_CORES):
        r = res.results[c]
        yc = r["y"].astype(np.float32)  # [16, I_TOTAL]
        # L0 scratch: [128, 8*28] -> tokens t=s*128+p, (b,k)=divmod(t,64)
        v0 = (
            r["Y0S"].astype(np.float32)
            .reshape(128, 8, 28).transpose(1, 0, 2).reshape(16, 64, 28)
        )
        yc[:, 0:1728] = v0[:, :, :27].reshape(16, 1728)
        yc[:, 1728:1792] = v0[:, :, 27]
        # L4 scratch: subtile 0 = tokens 0..120, subtile 1 = tokens 120..160
        v4r = r["Y4S"].astype(np.float32)  # [128, 514]
        v4 = np.concatenate(
            [v4r[:120, 0:257], v4r[:40, 257:514]], axis=0
        ).reshape(16, 10, 257)
        s4 = START[4]
        yc[:, s4 : s4 + 2560] = v4[:, :, :256].reshape(16, 2560)
        yc[:, s4 + 2560 : s4 + 2570] = v4[:, :, 256]
        # bias columns for L1/L2/L3
        ycol = r["YCOL"].astype(np.float32)  # [128, 80]
        for l in (1, 2, 3):
            ns = TOKL[l] // 128
            vals = ycol[:, COLBASE[l] : COLBASE[l] + ns].T.reshape(BPC, KN[l])
            cs = START[l] + KN[l] * KSCI[l]
            yc[:, cs : cs + KN[l]] = vals
        y[c * BPC : (c + 1) * BPC] = yc
    return y


def kernel(**inputs):
    global last_results
    if "nc" not in _cache:
        _cache["nc"] = _build_bass()
    nc = _cache["nc"]
    in_maps = _prep_inputs(inputs)
    res = run_bass_kernel_spmd(nc, in_maps, list(range(N_CORES)))
    last_results = res
    return _assemble(res)


# revision 10
# speedup vs baseline: 1.1510x; 1.0250x over previous
"""Trainium2 Bass kernel for nn_DebedderNeuronGroup_index.

Math (per layer l, kn=KN[l], ksci=KS[l]*CI[l], idim=ksci+1):
    out[b, k, o] = sum_d x[b, off_l + k, d] * W_l[o, d] + b_l[o]
    y[b, S_l + k*ksci + o] = out[b, k, o]        for o <  ksci
    y[b, S_l + kn*ksci + k] = out[b, k, ksci]    (bias-column tail block)

Strategy: pure data parallelism over batch (16 per core, 8 cores), bf16
matmuls (tokens stationary on PSUM partitions, o on the free dim), f16
stores. v2 scheduling, built from the v1 trace:
  - x is staged in HBM chunk-major ([128, 4*tl] per 1024-token chunk, c-major
    free dim) so every chunk load is 128 descriptors of 8KB instead of 512 of
    2KB; v1's 2KB descriptors capped the x queue at ~78GB/s and starved the
    PE for ~40us during L1/L2.
  - The bias column is folded into the main o-tiling (o-tiles cover idim =
    ksci+1, equal-split <=512), removing v1's separate column pass (~19us of
    PE streaming).  Per-subtile bias-column values are extracted to a
    [128, 80] SBUF buffer (scalar-engine copies), stored once, and scattered
    into y on the host (layout-only work).
  - L0/L4 outputs go to small DRAM scratches whole (host scatters them);
    their y-layout stores were ~1200 tiny (54-512B) descriptors that clog a
    DMA queue for ~25us.
  - L2 and L3 subtiles are interleaved 1:1 so the store stream is smooth
    (139GB/s) instead of a 378GB/s burst (L2) followed by 152GB/s (L3);
    stores are split across the scalar-ring queue (L1/L2) and gpsimd-ring
    queue (L3 halves), with WT3 preloaded on the vector-ring queue.
  - ~120 warmup matmuls on a zeroed tile keep the PE busy during the initial
    x DMA so the HAM clock gate ramps to full speed before real work.
"""

import numpy as np
import ml_dtypes

import concourse.bass as bass
import concourse.mybir as mybir
from concourse import bacc
from concourse.tile import TileContext
from concourse.bass_utils import run_bass_kernel_spmd

# ---------------------------------------------------------------- constants
N_CORES = 8
B = 128
BPC = B // N_CORES            # batches per core = 16
D = 512
KN = [64, 128, 256, 256, 10]
KSCI = [27, 576, 1152, 4096, 256]
IDIM = [k + 1 for k in KSCI]
START = [0, 1792, 75648, 370816, 1419648]
I_TOTAL = 1422218
TOKL = [BPC * k for k in KN]  # tokens per core per layer
NTOK = sum(TOKL)              # 11424
BBOFF = np.cumsum([0] + IDIM).tolist()  # bias table offset per layer
BBTOT = BBOFF[-1]             # 6112
TLOAD = 1024                  # tokens per x DMA chunk
BF16 = mybir.dt.bfloat16
F16 = mybir.dt.float16
F32 = mybir.dt.float32

# o-tile widths per layer: equal split of idim into ceil(idim/512) tiles
def _otw(idim):
    nt = -(-idim // 512)
    base, rem = divmod(idim, nt)
    return [base + 1] * rem + [base] * (nt - rem)
OTW = [_otw(i) for i in IDIM]

# x chunk schedule: (layer, t0, tl) in issue (= need) order
CH = [
    (0, 0, 1024), (4, 0, 160), (1, 0, 1024), (1, 1024, 1024),
    (2, 0, 1024), (2, 1024, 1024), (3, 0, 1024), (2, 2048, 1024),
    (3, 1024, 1024), (2, 3072, 1024), (3, 2048, 1024), (3, 3072, 1024),
]
CHIDX = {2: [4, 5, 7, 9], 3: [6, 8, 10, 11]}
XFOFF = np.cumsum([0] + [4 * tl for (_, _, tl) in CH]).tolist()
XFTOT = XFOFF[-1]             # 45696
# phase B subtile order: L2 solo x8, (L2,L3) pairs x24, L3 solo x8
SCHED = [(2, i) for i in range(8)]
for i in range(24):
    SCHED.append((2, 8 + i))
    SCHED.append((3, i))
SCHED += [(3, 24 + i) for i in range(8)]

# bias-column scratch layout: layer -> (col base, n subtiles)
COLBASE = {1: 0, 2: 16, 3: 48}
NCOLS = 80

_cache = {}
last_results = None


def _build_bass():
    nc = bacc.Bacc(
        "TRN2", target_bir_lowering=False, debug=False, num_devices=N_CORES
    )
    XF = nc.declare_dram_parameter("XF", [128, XFTOT], BF16, isOutput=False)
    WT = [
        nc.declare_dram_parameter(f"WT{l}", [128, 4 * IDIM[l]], BF16, isOutput=False)
        for l in range(5)
    ]
    BBp = nc.declare_dram_parameter("BB", [128, BBTOT], BF16, isOutput=False)
    y = nc.declare_dram_parameter("y", [BPC, I_TOTAL], F16, isOutput=True)
    Y0S = nc.declare_dram_parameter("Y0S", [128, 8 * 28], F16, isOutput=True)
    Y4S = nc.declare_dram_parameter("Y4S", [128, 2 * 257], F16, isOutput=True)
    YCOL = nc.declare_dram_parameter("YCOL", [128, NCOLS], F16, isOutput=True)

    with TileContext(nc) as tc:
        with (
            tc.tile_pool(name="wt", bufs=1) as wt_pool,
            tc.tile_pool(name="bias", bufs=1) as bias_pool,
            tc.tile_pool(name="x", bufs=5) as x_pool,
            tc.tile_pool(name="outL", bufs=4) as outL_pool,
            tc.tile_pool(name="out1", bufs=16) as out1_pool,
            tc.tile_pool(name="outS", bufs=8) as outS_pool,
            tc.tile_pool(name="fix", bufs=1) as fix_pool,
            tc.tile_pool(name="ps", bufs=6, space="PSUM") as ps_pool,
            tc.tile_pool(name="pw", bufs=1, space="PSUM") as pw_pool,
        ):
            # ---- PE warmup: keep the HAM activity monitor fed while the
            # first x chunk loads, so real matmuls start at full clock.
            warm = fix_pool.tile([128, 128], BF16, tag="warm")
            nc.vector.memset(warm[:, :], 0.0)
            wps = pw_pool.tile([128, 128], F32, tag="wps")
            for _ in range(64):
                nc.tensor.matmul(
                    out=wps[:, :], lhsT=warm[:, :], rhs=warm[:, :],
                    start=True, stop=True,
                )

            # ---- tables, all on the scalar-ring queue (stores don't start
            # until the small tables are through).  WT2/WT3 triggers are
            # placed later in the scalar instruction stream so their bytes
            # don't crowd out x/WT0/WT4/WT1 during the startup window.
            bb = bias_pool.tile([128, BBTOT], BF16, tag="bb")
            wt = {}

            def load_tables(l, eng):
                t = wt_pool.tile([128, 4 * IDIM[l]], BF16, tag=f"wt{l}")
                eng.dma_start(out=t[:, :], in_=WT[l][:, :])
                eng.dma_start(
                    out=bb[:, BBOFF[l] : BBOFF[l] + IDIM[l]],
                    in_=BBp[:, BBOFF[l] : BBOFF[l] + IDIM[l]],
                )
                wt[l] = t[:, :].rearrange("p (c o) -> p c o", c=4)

            for l in (0, 4, 1):
                load_tables(l, nc.scalar)

            colbuf = fix_pool.tile([128, NCOLS], F16, tag="colbuf")
            ob0 = fix_pool.tile([128, 8 * 28], F16, tag="ob0")
            ob4 = fix_pool.tile([128, 2 * 257], F16, tag="ob4")
            nc.vector.memset(ob4[:, :], 0.0)  # rows >= sl never written

            ymain = {
                l: y[:, START[l] : START[l] + KN[l] * KSCI[l]].rearrange(
                    "b (k o) -> b k o", o=KSCI[l]
                )
                for l in (1, 2, 3)
            }

            def load_chunk(ci):
                _, _, tl = CH[ci]
                xt = x_pool.tile([128, 4 * TLOAD], BF16, tag="xt")
                nc.sync.dma_start(
                    out=xt[:, : 4 * tl], in_=XF[:, XFOFF[ci] : XFOFF[ci] + 4 * tl]
                )
                return xt[:, : 4 * tl].rearrange("p (c t) -> p c t", c=4)

            def subtile_mm(l, xv, s0, sl, drain):
                o0 = 0
                for oi, no in enumerate(OTW[l]):
                    ps = ps_pool.tile([128, 512], F32, tag="ps")
                    for dc in range(4):
                        nc.tensor.matmul(
                            out=ps[:sl, :no],
                            lhsT=xv[:, dc, s0 : s0 + sl],
                            rhs=wt[l][:, dc, o0 : o0 + no],
                            start=(dc == 0),
                            stop=(dc == 3),
                        )
                    drain(oi, o0, no, ps)
                    o0 += no

            def badd(out_ap, ps, sl, no, l, o0):
                nc.vector.tensor_add(
                    out=out_ap,
                    in0=ps[:sl, :no],
                    in1=bb[:sl, BBOFF[l] + o0 : BBOFF[l] + o0 + no],
                )

            # ---- phase A: L0 -> scratch
            xv = load_chunk(0)
            for s in range(8):
                def dr0(oi, o0, no, ps, s=s):
                    badd(ob0[:128, s * 28 + o0 : s * 28 + o0 + no], ps, 128, no, 0, o0)
                subtile_mm(0, xv, s * 128, 128, dr0)
            nc.gpsimd.dma_start(out=Y0S[:, :], in_=ob0[:, :])

            # ---- L4 -> scratch (2 subtiles: 120 + 40 tokens)
            xv = load_chunk(1)
            for si, (s0, sl) in enumerate(((0, 120), (120, 40))):
                def dr4(oi, o0, no, ps, si=si, sl=sl):
                    badd(
                        ob4[:sl, si * 257 + o0 : si * 257 + o0 + no],
                        ps, sl, no, 4, o0,
                    )
                subtile_mm(4, xv, s0, sl, dr4)
            nc.gpsimd.dma_start(out=Y4S[:, :], in_=ob4[:, :])

            # ---- L1: 16 subtiles = 16 batches; stores alternate rings
            load_tables(2, nc.scalar)
            for ch in range(2):
                xv = load_chunk(2 + ch)
                for si in range(8):
                    s = ch * 8 + si
                    ob = out1_pool.tile([128, IDIM[1]], F16, tag="ob1")
                    def dr1(oi, o0, no, ps, ob=ob):
                        badd(ob[:128, o0 : o0 + no], ps, 128, no, 1, o0)
                    subtile_mm(1, xv, si * 128, 128, dr1)
                    eng = nc.scalar if s % 2 == 0 else nc.gpsimd
                    eng.dma_start(out=ymain[1][s, 0:128, :], in_=ob[:, : KSCI[1]])
                    nc.scalar.copy(
                        out=colbuf[:, COLBASE[1] + s : COLBASE[1] + s + 1],
                        in_=ob[:, KSCI[1] : IDIM[1]],
                    )
                    if s == 3:
                        # fires ~mid-L1 on the scalar ring: WT3's 5.25MB
                        # streams behind WT2 without starving phase-A loads
                        load_tables(3, nc.scalar)

            # ---- phase B: 8x L2 solo, 24x (L2,L3) pairs, 8x L3 solo
            xcur = {}
            for l, s in SCHED:
                if s % 8 == 0:
                    xcur[l] = load_chunk(CHIDX[l][s // 8])
                b0, k0 = divmod(s * 128, KN[l])
                if l == 2:
                    ob = outS_pool.tile([128, IDIM[2]], F16, tag="ob2")
                    def dr2(oi, o0, no, ps, ob=ob):
                        badd(ob[:128, o0 : o0 + no], ps, 128, no, 2, o0)
                    subtile_mm(2, xcur[2], (s % 8) * 128, 128, dr2)
                    eng = nc.scalar if s % 2 == 0 else nc.gpsimd
                    eng.dma_start(
                        out=ymain[2][b0, k0 : k0 + 128, :], in_=ob[:, : KSCI[2]]
                    )
                    nc.scalar.copy(
                        out=colbuf[:, COLBASE[2] + s : COLBASE[2] + s + 1],
                        in_=ob[:, KSCI[2] : IDIM[2]],
                    )
                else:
                    # L3 subtile; halves split across both store queues
                    enA = nc.scalar if s % 2 == 0 else nc.gpsimd
                    enB = nc.gpsimd if s % 2 == 0 else nc.scalar
                    ob3 = outL_pool.tile([128, IDIM[3]], F16, tag="ob3")
                    def dr3(oi, o0, no, ps, ob3=ob3, b0=b0, k0=k0, enA=enA):
                        badd(ob3[:128, o0 : o0 + no], ps, 128, no, 3, o0)
                        if oi == 4:  # columns [0, 2277) drained, store [0, 2048)
                            enA.dma_start(
                                out=ymain[3][b0, k0 : k0 + 128, 0:2048],
                                in_=ob3[:, 0:2048],
                            )
                    subtile_mm(3, xcur[3], (s % 8) * 128, 128, dr3)
                    enB.dma_start(
                        out=ymain[3][b0, k0 : k0 + 128, 2048:4096],
                        in_=ob3[:, 2048:4096],
                    )
                    nc.scalar.copy(
                        out=colbuf[:, COLBASE[3] + s : COLBASE[3] + s + 1],
                        in_=ob3[:, KSCI[3] : IDIM[3]],
                    )
            nc.gpsimd.dma_start(out=YCOL[:, :], in_=colbuf[:, :])
    nc.compile()
    return nc


def _prep_inputs(inputs):
    x = np.asarray(inputs["x"], dtype=np.float32)
    xb = x.astype(ml_dtypes.bfloat16)
    shared = {}
    for l in range(5):
        W = np.asarray(inputs[f"W{l}"], dtype=np.float32).astype(ml_dtypes.bfloat16)
        # [128, 4*idim] with free dim (c, o); row p, chunk c holds W.T[c*128+p]
        shared[f"WT{l}"] = np.ascontiguousarray(
            W.T.reshape(4, 128, IDIM[l]).transpose(1, 0, 2).reshape(128, 4 * IDIM[l])
        )
    bbvec = np.concatenate(
        [np.asarray(inputs[f"b{l}"], dtype=np.float32)[: IDIM[l]] for l in range(5)]
    )
    shared["BB"] = np.ascontiguousarray(
        np.broadcast_to(bbvec.astype(ml_dtypes.bfloat16), (128, BBTOT))
    )
    off = np.cumsum([0] + KN).tolist()
    in_maps = []
    for c in range(N_CORES):
        xc = xb[c * BPC : (c + 1) * BPC]  # [16, 714, 512] bf16
        xTl = [
            np.transpose(xc[:, off[l] : off[l] + KN[l]], (2, 0, 1)).reshape(D, -1)
            for l in range(5)
        ]
        parts = []
        for l, t0, tl in CH:
            blk = xTl[l][:, t0 : t0 + tl]  # [512, tl]
            parts.append(
                blk.reshape(4, 128, tl).transpose(1, 0, 2).reshape(128, 4 * tl)
            )
        in_maps.append({"XF": np.ascontiguousarray(np.concatenate(parts, axis=1)),
                        **shared})
    return in_maps


def _assemble(res):
    y = np.empty((B, I_TOTAL), np.float32)
    for c in range(N_CORES):
        r = res.results[c]
        yc = r["y"].astype(np.float32)  # [16, I_TOTAL]
        # L0 scratch: [128, 8*28] -> tokens t=s*128+p, (b,k)=divmod(t,64)
        v0 = (
            r["Y0S"].astype(np.float32)
            .reshape(128, 8, 28).transpose(1, 0, 2).reshape(16, 64, 28)
        )
        yc[:, 0:1728] = v0[:, :, :27].reshape(16, 1728)
        yc[:, 1728:1792] = v0[:, :, 27]
        # L4 scratch: subtile 0 = tokens 0..120, subtile 1 = tokens 120..160
        v4r = r["Y4S"].astype(np.float32)  # [128, 514]
        v4 = np.concatenate(
            [v4r[:120, 0:257], v4r[:40, 257:514]], axis=0
        ).reshape(16, 10, 257)
        s4 = START[4]
        yc[:, s4 : s4 + 2560] = v4[:, :, :256].reshape(16, 2560)
        yc[:, s4 + 2560 : s4 + 2570] = v4[:, :, 256]
        # bias columns for L1/L2/L3
        ycol = r["YCOL"].astype(np.float32)  # [128, 80]
        for l in (1, 2, 3):
            ns = TOKL[l] // 128
            vals = ycol[:, COLBASE[l] : COLBASE[l] + ns].T.reshape(BPC, KN[l])
            cs = START[l] + KN[l] * KSCI[l]
            yc[:, cs : cs + KN[l]] = vals
        y[c * BPC : (c + 1) * BPC] = yc
    return y


def kernel(**inputs):
    global last_results
    if "nc" not in _cache:
        _cache["nc"] = _build_bass()
    nc = _cache["nc"]
    in_maps = _prep_inputs(inputs)
    res = run_bass_kernel_spmd(nc, in_maps, list(range(N_CORES)))
    last_results = res
    return _assemble(res)
